# revision 15
# baseline (speedup 1.0000x reference)
"""Trainium2 Bass kernel for BeliefPropagationCV (LDPC check-node update).

Math: out[b,o] = 2*atanh(clip(prod_i (mask[o,i]*x[b,i] + 1-mask[o,i])))

The product over masked entries is computed in log-domain so it becomes one
matmul (N=256) over the Tanner-graph mask per 128-row chunk of i:
    L[o,b] = sum_i mask[o,i] * ln|x[b,i]|     (matmul cols 0:128)
    N[o,b] = sum_i mask[o,i] * (x[b,i] < 0)   (matmul cols 128:256)
    t      = exp(L);  sgn = (-1)^N
    out    = sgn * (ln(1+t) - ln((1+eps) - t))
The (1+eps) bias on the second Ln reproduces the reference's clip exactly:
f32(1+1e-7) - 1.0 == 1.0 - f32(1-1e-7) == 1.1920929e-7, so zero-connection
rows (t==1) yield the same +-16.64 the reference produces, with no extra
min() op.

Layouts (host-side prep, no math): x is shipped pre-transposed into
chunk-column layout (x_cc[:, c*128+b] = x[b, c*128+p]) so the kernel needs
no PE transposes; the static Tanner mask ships as fp8e4m3 bits (0/1 exact)
in the same chunk-column layout, used directly as matmul weights (fp8
stationary x fp16 moving). Output leaves the device as fp16 (rel err
~5e-4 of scale, well under tolerance) and is upcast on host.

Sharding: output-dim (check-node rows) across 8 cores; each core reads the
full x_cc (1MB) + its mask shard (0.25MB) and writes outT [128(o),128(b)].

Engine split per iteration: x^2 on Pool (tensor_tensor mult, 3/4) + ACT
(Square, 1/4) -- walrus rejects tensor_scalar on Pool, so no bitand-abs
there; ACT Ln(x^2) -> fp16 rhs with the 1/2 folded into Exp(scale=0.5);
DVE sign indicators -> fp16 rhs; PE 16 accumulating matmuls; ACT+DVE
epilogue; Pool-issued output DMA (SP would head-block its load queue on
the epilogue wait). The timing loop uses For_i_pipelined (2 stages,
unroll=8, staged_num_bufs=4, staggered_reset) so iterations overlap with
no all-engine barrier: plain For_i's per-iteration barrier was the
dominant cost of the previous version (~17us -> ~6us).

Measurement note: per-call wall noise is ~+-20ms, so the loop-delta
timing needs >=60k iteration deltas to be trustworthy.
"""

import os
import sys
from contextlib import ExitStack

import numpy as np

for _p in ("/opt/trn_rl_repo", "/root/.axon_site/_ro/trn_rl_repo"):
    if os.path.isdir(_p) and _p not in sys.path:
        sys.path.append(_p)

import concourse.bacc as bacc
import concourse.bass as bass
import concourse.tile as tile
from concourse import mybir
from concourse.bass_utils import run_bass_kernel_spmd
from concourse.hw_specs import get_activation_tables

N_CORES = 8
B = 128          # batch
O = 1024         # check nodes (mask rows)
I = 2048         # variable-node messages (mask cols)
OS = O // N_CORES  # mask rows per core

F32 = mybir.dt.float32
FP16 = mybir.dt.float16
FP8 = mybir.dt.float8e4
I32 = mybir.dt.int32
U8 = mybir.dt.uint8
AF = mybir.ActivationFunctionType
ALU = mybir.AluOpType

N_CHUNKS = I // 128  # 16 k-chunks of 128
HALF = I // 2
POOL_SQ = 1536     # cols of x^2 on Pool (0.42 eff); rest on ACT Square
# f32(1 + 1e-7); BIAS1P - 1.0f == 1.0f - f32(1 - 1e-7) == 1.1920929e-7
BIAS1P = float(np.float32(1.0) + np.float32(1e-7))


def _emit_load(nc, xc, mk, x_d, m_d):
    # Split input loads across both hwdge queues (SP and ACT) so the two
    # x halves transfer in parallel; issue cost on ACT is just a doorbell.
    nc.sync.dma_start(xc[:, 0:HALF], x_d[:, 0:HALF])
    nc.scalar.dma_start(xc[:, HALF:I], x_d[:, HALF:I])
    nc.sync.dma_start(mk[:], m_d[:])


def _emit_compute(nc, tl, o_d):
    """tl: dict of tiles. Emits abs/ln/islt, 16 matmuls, epilogue, out DMA."""
    xc, mk, ax, rhs, po = tl["xc"], tl["mk"], tl["ax"], tl["rhs"], tl["po"]
    rhs3 = rhs[:].rearrange("p (c n) -> p c n", n=256)
    xc3 = xc[:].rearrange("p (c n) -> p c n", n=128)
    ax3 = ax[:].rearrange("p (c n) -> p c n", n=128)

    # x^2 (Ln(x^2)=2ln|x|, halved later in Exp's scale). Measured on HW:
    # Pool's software tensor_tensor runs at ~0.2 of roofline, so it gets
    # only a small slice; DVE takes the first Ln-half, ACT+Pool the second.
    nc.vector.tensor_tensor(ax[:, 0:HALF], xc[:, 0:HALF],
                            xc[:, 0:HALF], ALU.mult)
    nc.scalar.activation(ax[:, HALF:HALF + 768], xc[:, HALF:HALF + 768],
                         AF.Square)
    nc.gpsimd.tensor_tensor(ax[:, HALF + 768:I], xc[:, HALF + 768:I],
                            xc[:, HALF + 768:I], ALU.mult)
    for h in range(2):
        cs = slice(h * 8, (h + 1) * 8)
        # negative-factor indicators on DVE (reads signed x directly)
        nc.vector.tensor_scalar(rhs3[:, cs, 128:256], xc3[:, cs, :],
                                0.0, None, ALU.is_lt)
        # ln(x^2) on ACT, fp16 straight into the matmul moving operand
        nc.scalar.activation(rhs3[:, cs, 0:128], ax3[:, cs, :], AF.Ln)

    mk8 = mk[:].bitcast(FP8)
    for c in range(N_CHUNKS):
        nc.tensor.matmul(
            po[:], mk8[:, c * 128:(c + 1) * 128], rhs3[:, c, :],
            start=(c == 0), stop=(c == N_CHUNKS - 1))

    # Epilogue on [128(o), 128(b)]: po[:,0:128]=L, po[:,128:256]=N.
    t, a, bb = tl["t"], tl["a"], tl["bb"]
    pari, par2, u, ot = tl["pari"], tl["par2"], tl["u"], tl["ot"]
    nc.scalar.activation(t[:], po[:, 0:128], AF.Exp, scale=0.5)
    nc.vector.tensor_copy(pari[:], po[:, 128:256])  # f32 count -> int32 exact
    nc.scalar.activation(a[:], t[:], AF.Ln, bias=1.0)            # ln(1+t)
    nc.scalar.activation(bb[:], t[:], AF.Ln, bias=BIAS1P, scale=-1.0)
    # sgn = 1 - 2*(N & 1): bitwise and arith ALU ops can't mix in one instr
    par = tl["par"]
    nc.vector.tensor_scalar(par[:], pari[:], 1, None, ALU.bitwise_and)
    nc.vector.tensor_scalar(par2[:], par[:], -2.0, 1.0, ALU.mult, ALU.add)
    # final combine on Pool (idle, feeds its own output DMA directly)
    nc.gpsimd.tensor_tensor(u[:], a[:], bb[:], ALU.subtract)
    nc.gpsimd.tensor_tensor(ot[:], u[:], par2[:], ALU.mult)
    # outT DMA issues from Pool: SP would head-block its own in-order queue
    # on the epilogue wait, gating the next tick's input loads.
    nc.gpsimd.dma_start(o_d[:], ot[:])


_SMALL = (("t", F32), ("a", F32), ("bb", F32), ("pari", I32), ("par", I32),
          ("par2", F32), ("u", F32), ("ot", FP16))


def build(loop_n: int = 0) -> bass.Bass:
    """Build the SPMD program. loop_n>0 wraps the body in a pipelined loop."""
    nc = bacc.Bacc("TRN2", target_bir_lowering=False, debug=False,
                   num_devices=N_CORES)
    # Register the clip bias as a const AP (same recipe as Bass init consts).
    bias_t = nc.alloc_sbuf_tensor("const-bias1p", [128, 1], F32)
    nc.gpsimd.memset(bias_t.ap(), BIAS1P)
    nc.const_aps.aps[(F32, BIAS1P)] = bias_t.ap()
    nc.all_engine_barrier()
    x_d = nc.dram_tensor("x", [B, I], F32, kind="ExternalInput").ap()
    m_d = nc.dram_tensor("mask", [128, I], U8, kind="ExternalInput").ap()
    o_d = nc.dram_tensor("outT", [OS, B], FP16, kind="ExternalOutput").ap()
    with tile.TileContext(nc) as tc:
        with ExitStack() as ctx:
            # One table set (natural_log_exp_and_others) covers Ln+Exp; a
            # single pre-loop load means the insertion pass adds none inside.
            set_id = [i for i, (n, _) in enumerate(
                get_activation_tables(nc.m.arch).items())
                if n == "natural_log_exp_and_others"][0]
            nc.scalar.add_instruction(mybir.InstLoadActFuncSet(
                name=nc.get_next_instruction_name(), ins=[], outs=[],
                act_func_set_id=set_id))

            sb = ctx.enter_context(tc.tile_pool(name="sb", bufs=1))
            ps = ctx.enter_context(tc.tile_pool(name="ps", bufs=1,
                                                space="PSUM"))
            if loop_n == 0:
                tl = {
                    "xc": sb.tile([128, I], F32, name="xc"),
                    "mk": sb.tile([128, I], U8, name="mk"),
                    "ax": sb.tile([128, I], F32, name="ax"),
                    "rhs": sb.tile([128, N_CHUNKS * 256], FP16, name="rhs"),
                    "po": ps.tile([128, 256], F32, name="po"),
                }
                for nm, dt in _SMALL:
                    tl[nm] = sb.tile([128, B], dt, name=nm)
                _emit_load(nc, tl["xc"], tl["mk"], x_d, m_d)
                _emit_compute(nc, tl, o_d)
            else:
                po_ring = [ps.tile([128, 256], F32, name=f"po{i}")
                           for i in range(4)]

                def load(pipe, iv):
                    xc = pipe.intermediate_tile([128, I], F32, name="xc")
                    mk = pipe.intermediate_tile([128, I], U8, name="mk")
                    _emit_load(nc, xc, mk, x_d, m_d)
                    return (xc, mk)

                def compute(pipe, iv, ins):
                    xc, mk = ins
                    tl = {"xc": xc, "mk": mk}
                    tl["ax"] = pipe.intermediate_tile([128, I], F32,
                                                      name="ax")
                    tl["rhs"] = pipe.intermediate_tile(
                        [128, N_CHUNKS * 256], FP16, name="rhs")
                    tl["po"] = pipe.intermediate_tile(
                        [128, 256], F32, name="po", bufs=4,
                        prealloc=po_ring)
                    for nm, dt in _SMALL:
                        tl[nm] = pipe.intermediate_tile([128, B], dt,
                                                        name=nm)
                    _emit_compute(nc, tl, o_d)

                tc.For_i_pipelined([load, compute], 0, loop_n,
                                   unroll=8, staged_num_bufs=4,
                                   staggered_reset=True)
    nc.compile()
    return nc


def _prep_x(x: np.ndarray) -> np.ndarray:
    """Chunk-column transpose: x_cc[p, c*128+b] = x[b, c*128+p]."""
    xt = np.ascontiguousarray(x.T).reshape(N_CHUNKS, 128, 128)
    return np.ascontiguousarray(
        xt.transpose(1, 0, 2).reshape(128, I)).astype(np.float32)


def _prep_mask(shard: np.ndarray) -> np.ndarray:
    """fp8e4m3 bit pattern (1.0 -> 0x38) in chunk-column layout, uint8."""
    st = np.ascontiguousarray(shard.T).reshape(N_CHUNKS, 128, 128)
    cc = st.transpose(1, 0, 2).reshape(128, I)
    return np.ascontiguousarray((cc > 0).astype(np.uint8) * np.uint8(0x38))


_CACHE: dict = {}


def kernel(x: np.ndarray, mask: np.ndarray) -> np.ndarray:
    nc = _CACHE.get("nc")
    if nc is None:
        nc = _CACHE["nc"] = build()
    x = np.ascontiguousarray(np.asarray(x), dtype=np.float32)
    mask = np.ascontiguousarray(np.asarray(mask), dtype=np.float32)
    x_cc = _prep_x(x)
    in_maps = []
    for c in range(N_CORES):
        in_maps.append({"x": x_cc,
                        "mask": _prep_mask(mask[c * OS:(c + 1) * OS])})
    res = run_bass_kernel_spmd(nc, in_maps, list(range(N_CORES)))
    out = np.concatenate(
        [np.asarray(res.results[c]["outT"]).T for c in range(N_CORES)],
        axis=1)  # [B, O]
    return np.ascontiguousarray(out.astype(np.float32))


# revision 17
# speedup vs baseline: 1.0222x; 1.0222x over previous
"""Trainium2 Bass kernel for BeliefPropagationCV (LDPC check-node update).

Math: out[b,o] = 2*atanh(clip(prod_i (mask[o,i]*x[b,i] + 1-mask[o,i])))

The product over masked entries is computed in log-domain so it becomes one
matmul (N=256) over the Tanner-graph mask per 128-row chunk of i:
    L[o,b] = sum_i mask[o,i] * ln|x[b,i]|     (matmul cols 0:128)
    N[o,b] = sum_i mask[o,i] * (x[b,i] < 0)   (matmul cols 128:256)
    t      = exp(L);  sgn = (-1)^N
    out    = sgn * (ln(1+t) - ln((1+eps) - t))
The (1+eps) bias on the second Ln reproduces the reference's clip exactly:
f32(1+1e-7) - 1.0 == 1.0 - f32(1-1e-7) == 1.1920929e-7, so zero-connection
rows (t==1) yield the same +-16.64 the reference produces, with no extra
min() op.

Layouts (host-side prep, no math): x is shipped pre-transposed into
chunk-column layout (x_cc[:, c*128+b] = x[b, c*128+p]) so the kernel needs
no PE transposes; the static Tanner mask ships as fp8e4m3 bits (0/1 exact)
in the same chunk-column layout, used directly as matmul weights (fp8
stationary x fp16 moving). Output leaves the device as fp16 (rel err
~5e-4 of scale, well under tolerance) and is upcast on host.

Sharding: output-dim (check-node rows) across 8 cores; each core reads the
full x_cc (1MB) + its mask shard (0.25MB) and writes outT [128(o),128(b)].

Engine split per iteration: x^2 on DVE (cols 0:1024, tensor_tensor mult)
+ ACT (Square, 1024:1792) + Pool (1792:2048) -- walrus rejects
tensor_scalar on Pool entirely, and Pool's software tensor_tensor runs
at only ~0.2 of roofline on HW, so it gets a small slice; ACT Ln(x^2) ->
fp16 rhs with the 1/2 folded into Exp(scale=0.5); DVE sign indicators ->
fp16 rhs; PE 16 accumulating matmuls; ACT+DVE epilogue; Pool-issued
output DMA (SP would head-block its load queue on the epilogue wait).
The timing loop uses For_i_pipelined (2 stages, unroll=8,
staged_num_bufs=4, staggered_reset) so iterations overlap with no
all-engine barrier: plain For_i's per-iteration barrier was the dominant
cost of the previous version (~17us with barriers -> 7.4us pipelined ->
6.4us after moving x^2 off Pool). An independent-chains probe (all
engines loaded, no cross-deps) measured a 6.8us engine floor, so the
remaining time is engine-busy, not overlap loss.

Measurement note: per-call wall noise is ~+-20ms, so the loop-delta
timing needs >=60k iteration deltas to be trustworthy.
"""

import os
import sys
from contextlib import ExitStack

import numpy as np

for _p in ("/opt/trn_rl_repo", "/root/.axon_site/_ro/trn_rl_repo"):
    if os.path.isdir(_p) and _p not in sys.path:
        sys.path.append(_p)

import concourse.bacc as bacc
import concourse.bass as bass
import concourse.tile as tile
from concourse import mybir
from concourse.bass_utils import run_bass_kernel_spmd
from concourse.hw_specs import get_activation_tables

N_CORES = 8
B = 128          # batch
O = 1024         # check nodes (mask rows)
I = 2048         # variable-node messages (mask cols)
OS = O // N_CORES  # mask rows per core

F32 = mybir.dt.float32
FP16 = mybir.dt.float16
FP8 = mybir.dt.float8e4
I32 = mybir.dt.int32
U8 = mybir.dt.uint8
AF = mybir.ActivationFunctionType
ALU = mybir.AluOpType

N_CHUNKS = I // 128  # 16 k-chunks of 128
HALF = I // 2
POOL_SQ = 1536     # cols of x^2 on Pool (0.42 eff); rest on ACT Square
# f32(1 + 1e-7); BIAS1P - 1.0f == 1.0f - f32(1 - 1e-7) == 1.1920929e-7
BIAS1P = float(np.float32(1.0) + np.float32(1e-7))


def _emit_load(nc, xc, mk, x_d, m_d):
    nc.sync.dma_start(xc[:, 0:HALF], x_d[:, 0:HALF])
    nc.sync.dma_start(xc[:, HALF:I], x_d[:, HALF:I])
    nc.sync.dma_start(mk[:], m_d[:])


def _emit_compute(nc, tl, o_d):
    """tl: dict of tiles. Emits abs/ln/islt, 16 matmuls, epilogue, out DMA."""
    xc, mk, ax, rhs, po = tl["xc"], tl["mk"], tl["ax"], tl["rhs"], tl["po"]
    rhs3 = rhs[:].rearrange("p (c n) -> p c n", n=256)
    xc3 = xc[:].rearrange("p (c n) -> p c n", n=128)
    ax3 = ax[:].rearrange("p (c n) -> p c n", n=128)

    # x^2 (Ln(x^2)=2ln|x|, halved later in Exp's scale). Measured on HW:
    # Pool's software tensor_tensor runs at ~0.2 of roofline, so it gets
    # only a small slice; DVE takes the first Ln-half, ACT+Pool the second.
    nc.vector.tensor_tensor(ax[:, 0:HALF], xc[:, 0:HALF],
                            xc[:, 0:HALF], ALU.mult)
    nc.scalar.activation(ax[:, HALF:HALF + 768], xc[:, HALF:HALF + 768],
                         AF.Square)
    nc.gpsimd.tensor_tensor(ax[:, HALF + 768:I], xc[:, HALF + 768:I],
                            xc[:, HALF + 768:I], ALU.mult)
    for h in range(2):
        cs = slice(h * 8, (h + 1) * 8)
        # negative-factor indicators on DVE (reads signed x directly)
        nc.vector.tensor_scalar(rhs3[:, cs, 128:256], xc3[:, cs, :],
                                0.0, None, ALU.is_lt)
        # ln(x^2) on ACT, fp16 straight into the matmul moving operand
        nc.scalar.activation(rhs3[:, cs, 0:128], ax3[:, cs, :], AF.Ln)

    mk8 = mk[:].bitcast(FP8)
    for c in range(N_CHUNKS):
        nc.tensor.matmul(
            po[:], mk8[:, c * 128:(c + 1) * 128], rhs3[:, c, :],
            start=(c == 0), stop=(c == N_CHUNKS - 1))

    # Epilogue on [128(o), 128(b)]: po[:,0:128]=L, po[:,128:256]=N.
    t, a, bb = tl["t"], tl["a"], tl["bb"]
    pari, par2, u, ot = tl["pari"], tl["par2"], tl["u"], tl["ot"]
    nc.scalar.activation(t[:], po[:, 0:128], AF.Exp, scale=0.5)
    nc.vector.tensor_copy(pari[:], po[:, 128:256])  # f32 count -> int32 exact
    nc.scalar.activation(a[:], t[:], AF.Ln, bias=1.0)            # ln(1+t)
    nc.scalar.activation(bb[:], t[:], AF.Ln, bias=BIAS1P, scale=-1.0)
    # sgn = 1 - 2*(N & 1): bitwise and arith ALU ops can't mix in one instr
    par = tl["par"]
    nc.vector.tensor_scalar(par[:], pari[:], 1, None, ALU.bitwise_and)
    nc.vector.tensor_scalar(par2[:], par[:], -2.0, 1.0, ALU.mult, ALU.add)
    nc.vector.tensor_sub(u[:], a[:], bb[:])
    nc.vector.tensor_mul(ot[:], u[:], par2[:])
    # outT DMA issues from Pool: SP would head-block its own in-order queue
    # on the epilogue wait, gating the next tick's input loads.
    nc.gpsimd.dma_start(o_d[:], ot[:])


_SMALL = (("t", F32), ("a", F32), ("bb", F32), ("pari", I32), ("par", I32),
          ("par2", F32), ("u", F32), ("ot", FP16))


def build(loop_n: int = 0) -> bass.Bass:
    """Build the SPMD program. loop_n>0 wraps the body in a pipelined loop."""
    nc = bacc.Bacc("TRN2", target_bir_lowering=False, debug=False,
                   num_devices=N_CORES)
    # Register the clip bias as a const AP (same recipe as Bass init consts).
    bias_t = nc.alloc_sbuf_tensor("const-bias1p", [128, 1], F32)
    nc.gpsimd.memset(bias_t.ap(), BIAS1P)
    nc.const_aps.aps[(F32, BIAS1P)] = bias_t.ap()
    nc.all_engine_barrier()
    x_d = nc.dram_tensor("x", [B, I], F32, kind="ExternalInput").ap()
    m_d = nc.dram_tensor("mask", [128, I], U8, kind="ExternalInput").ap()
    o_d = nc.dram_tensor("outT", [OS, B], FP16, kind="ExternalOutput").ap()
    with tile.TileContext(nc) as tc:
        with ExitStack() as ctx:
            # One table set (natural_log_exp_and_others) covers Ln+Exp; a
            # single pre-loop load means the insertion pass adds none inside.
            set_id = [i for i, (n, _) in enumerate(
                get_activation_tables(nc.m.arch).items())
                if n == "natural_log_exp_and_others"][0]
            nc.scalar.add_instruction(mybir.InstLoadActFuncSet(
                name=nc.get_next_instruction_name(), ins=[], outs=[],
                act_func_set_id=set_id))

            sb = ctx.enter_context(tc.tile_pool(name="sb", bufs=1))
            ps = ctx.enter_context(tc.tile_pool(name="ps", bufs=1,
                                                space="PSUM"))
            if loop_n == 0:
                tl = {
                    "xc": sb.tile([128, I], F32, name="xc"),
                    "mk": sb.tile([128, I], U8, name="mk"),
                    "ax": sb.tile([128, I], F32, name="ax"),
                    "rhs": sb.tile([128, N_CHUNKS * 256], FP16, name="rhs"),
                    "po": ps.tile([128, 256], F32, name="po"),
                }
                for nm, dt in _SMALL:
                    tl[nm] = sb.tile([128, B], dt, name=nm)
                _emit_load(nc, tl["xc"], tl["mk"], x_d, m_d)
                _emit_compute(nc, tl, o_d)
            else:
                po_ring = [ps.tile([128, 256], F32, name=f"po{i}")
                           for i in range(4)]

                def load(pipe, iv):
                    xc = pipe.intermediate_tile([128, I], F32, name="xc")
                    mk = pipe.intermediate_tile([128, I], U8, name="mk")
                    _emit_load(nc, xc, mk, x_d, m_d)
                    return (xc, mk)

                def compute(pipe, iv, ins):
                    xc, mk = ins
                    tl = {"xc": xc, "mk": mk}
                    tl["ax"] = pipe.intermediate_tile([128, I], F32,
                                                      name="ax")
                    tl["rhs"] = pipe.intermediate_tile(
                        [128, N_CHUNKS * 256], FP16, name="rhs")
                    tl["po"] = pipe.intermediate_tile(
                        [128, 256], F32, name="po", bufs=4,
                        prealloc=po_ring)
                    for nm, dt in _SMALL:
                        tl[nm] = pipe.intermediate_tile([128, B], dt,
                                                        name=nm)
                    _emit_compute(nc, tl, o_d)

                tc.For_i_pipelined([load, compute], 0, loop_n,
                                   unroll=8, staged_num_bufs=4,
                                   staggered_reset=True)
    nc.compile()
    return nc


def _prep_x(x: np.ndarray) -> np.ndarray:
    """Chunk-column transpose: x_cc[p, c*128+b] = x[b, c*128+p]."""
    xt = np.ascontiguousarray(x.T).reshape(N_CHUNKS, 128, 128)
    return np.ascontiguousarray(
        xt.transpose(1, 0, 2).reshape(128, I)).astype(np.float32)


def _prep_mask(shard: np.ndarray) -> np.ndarray:
    """fp8e4m3 bit pattern (1.0 -> 0x38) in chunk-column layout, uint8."""
    st = np.ascontiguousarray(shard.T).reshape(N_CHUNKS, 128, 128)
    cc = st.transpose(1, 0, 2).reshape(128, I)
    return np.ascontiguousarray((cc > 0).astype(np.uint8) * np.uint8(0x38))


_CACHE: dict = {}


def kernel(x: np.ndarray, mask: np.ndarray) -> np.ndarray:
    nc = _CACHE.get("nc")
    if nc is None:
        nc = _CACHE["nc"] = build()
    x = np.ascontiguousarray(np.asarray(x), dtype=np.float32)
    mask = np.ascontiguousarray(np.asarray(mask), dtype=np.float32)
    x_cc = _prep_x(x)
    in_maps = []
    for c in range(N_CORES):
        in_maps.append({"x": x_cc,
                        "mask": _prep_mask(mask[c * OS:(c + 1) * OS])})
    res = run_bass_kernel_spmd(nc, in_maps, list(range(N_CORES)))
    out = np.concatenate(
        [np.asarray(res.results[c]["outT"]).T for c in range(N_CORES)],
        axis=1)  # [B, O]
    return np.ascontiguousarray(out.astype(np.float32))


# revision 20
# speedup vs baseline: 1.0286x; 1.0062x over previous
"""Trainium2 Bass kernel for BeliefPropagationCV (LDPC check-node update).

Math: out[b,o] = 2*atanh(clip(prod_i (mask[o,i]*x[b,i] + 1-mask[o,i])))

The product over masked entries is computed in log-domain so it becomes one
matmul (N=256) over the Tanner-graph mask per 128-row chunk of i:
    L[o,b] = sum_i mask[o,i] * ln|x[b,i]|     (matmul cols 0:128)
    N[o,b] = sum_i mask[o,i] * (x[b,i] < 0)   (matmul cols 128:256)
    t      = exp(L);  sgn = (-1)^N
    out    = sgn * (ln(1+t) - ln((1+eps) - t))
The (1+eps) bias on the second Ln reproduces the reference's clip exactly:
f32(1+1e-7) - 1.0 == 1.0 - f32(1-1e-7) == 1.1920929e-7, so zero-connection
rows (t==1) yield the same +-16.64 the reference produces, with no extra
min() op.

Layouts (host-side prep, no math): x is shipped pre-transposed into
chunk-column layout (x_cc[:, c*128+b] = x[b, c*128+p]) so the kernel needs
no PE transposes; the static Tanner mask ships as fp8e4m3 bits (0/1 exact)
in the same chunk-column layout, used directly as matmul weights (fp8
stationary x fp16 moving). Output leaves the device as fp16 (rel err
~5e-4 of scale, well under tolerance) and is upcast on host.

Sharding: output-dim (check-node rows) across 8 cores; each core reads the
full x_cc (1MB) + its mask shard (0.25MB) and writes outT [128(o),128(b)].

Engine split per iteration: x^2 on DVE (cols 0:1024, tensor_tensor mult)
+ ACT (Square, 1024:1792) + Pool (1792:2048) -- walrus rejects
tensor_scalar on Pool entirely, and Pool's software tensor_tensor runs
at only ~0.2 of roofline on HW, so it gets a small slice; ACT Ln(x^2) ->
fp16 rhs with the 1/2 folded into Exp(scale=0.5); DVE sign indicators ->
fp16 rhs; PE 16 accumulating matmuls; ACT+DVE epilogue; Pool-issued
output DMA (SP would head-block its load queue on the epilogue wait).
The timing loop uses For_i_pipelined (2 stages, unroll=8,
staged_num_bufs=4, staggered_reset) so iterations overlap with no
all-engine barrier: plain For_i's per-iteration barrier was the dominant
cost of the previous version (~17us with barriers -> 7.4us pipelined ->
6.4us after moving x^2 off Pool). An independent-chains probe (all
engines loaded, no cross-deps) measured a 6.8us engine floor, so the
remaining time is engine-busy, not overlap loss.

Measurement note: per-call wall noise is ~+-20ms, so the loop-delta
timing needs >=60k iteration deltas to be trustworthy.
"""

import os
import sys
from contextlib import ExitStack

import numpy as np

for _p in ("/opt/trn_rl_repo", "/root/.axon_site/_ro/trn_rl_repo"):
    if os.path.isdir(_p) and _p not in sys.path:
        sys.path.append(_p)

import concourse.bacc as bacc
import concourse.bass as bass
import concourse.tile as tile
from concourse import mybir
from concourse.bass_utils import run_bass_kernel_spmd
from concourse.hw_specs import get_activation_tables

N_CORES = 8
B = 128          # batch
O = 1024         # check nodes (mask rows)
I = 2048         # variable-node messages (mask cols)
OS = O // N_CORES  # mask rows per core

F32 = mybir.dt.float32
FP16 = mybir.dt.float16
FP8 = mybir.dt.float8e4
I32 = mybir.dt.int32
U8 = mybir.dt.uint8
AF = mybir.ActivationFunctionType
ALU = mybir.AluOpType

N_CHUNKS = I // 128  # 16 k-chunks of 128
HALF = I // 2
POOL_SQ = 1536     # cols of x^2 on Pool (0.42 eff); rest on ACT Square
# f32(1 + 1e-7); BIAS1P - 1.0f == 1.0f - f32(1 - 1e-7) == 1.1920929e-7
BIAS1P = float(np.float32(1.0) + np.float32(1e-7))


def _emit_load(nc, xc, mk, x_d, m_d):
    nc.sync.dma_start(xc[:, 0:HALF], x_d[:, 0:HALF])
    nc.sync.dma_start(xc[:, HALF:I], x_d[:, HALF:I])
    nc.sync.dma_start(mk[:], m_d[:])


def _emit_compute(nc, tl, o_d):
    """tl: dict of tiles. Emits sq/ln/islt, 32 matmuls, epilogue, out DMA."""
    xc, mk, ax = tl["xc"], tl["mk"], tl["ax"]
    rl, rn, poL, poN = tl["rl"], tl["rn"], tl["poL"], tl["poN"]

    # x^2 (Ln(x^2)=2ln|x|, halved later in Exp's scale). Measured on HW:
    # Pool's software tensor_tensor runs at ~0.2 of roofline, so it gets
    # only a small slice; DVE takes the first Ln-half, ACT+Pool the second.
    nc.vector.tensor_tensor(ax[:, 0:HALF], xc[:, 0:HALF],
                            xc[:, 0:HALF], ALU.mult)
    nc.scalar.activation(ax[:, HALF:HALF + 768], xc[:, HALF:HALF + 768],
                         AF.Square)
    nc.gpsimd.tensor_tensor(ax[:, HALF + 768:I], xc[:, HALF + 768:I],
                            xc[:, HALF + 768:I], ALU.mult)
    for h in range(2):
        sl = slice(h * HALF, (h + 1) * HALF)
        # negative-factor indicators on DVE (reads signed x directly);
        # contiguous fp16 writes (strided chunk-interleaved writes were
        # suspected to cost DVE/ACT extra)
        nc.vector.tensor_scalar(rn[:, sl], xc[:, sl], 0.0, None, ALU.is_lt)
        # ln(x^2) on ACT, fp16 straight into the matmul moving operand
        nc.scalar.activation(rl[:, sl], ax[:, sl], AF.Ln)

    # Two N=128 matmuls per chunk (same fp8 weights) into separate PSUM
    # accumulators; the epilogue then reads L and N from different tiles,
    # avoiding cross-engine same-tile PSUM read serialization.
    mk8 = mk[:].bitcast(FP8)
    for c in range(N_CHUNKS):
        ks = slice(c * 128, (c + 1) * 128)
        nc.tensor.matmul(poL[:], mk8[:, ks], rl[:, ks],
                         start=(c == 0), stop=(c == N_CHUNKS - 1))
        nc.tensor.matmul(poN[:], mk8[:, ks], rn[:, ks],
                         start=(c == 0), stop=(c == N_CHUNKS - 1))

    # Epilogue on [128(o), 128(b)]: poL=L2, poN=N.
    t, a, bb = tl["t"], tl["a"], tl["bb"]
    pari, par2, u, ot = tl["pari"], tl["par2"], tl["u"], tl["ot"]
    nc.scalar.activation(t[:], poL[:], AF.Exp, scale=0.5)
    nc.vector.tensor_copy(pari[:], poN[:])  # f32 count -> int32 exact
    nc.scalar.activation(a[:], t[:], AF.Ln, bias=1.0)            # ln(1+t)
    nc.scalar.activation(bb[:], t[:], AF.Ln, bias=BIAS1P, scale=-1.0)
    # sgn = 1 - 2*(N & 1): bitwise and arith ALU ops can't mix in one instr
    par = tl["par"]
    nc.vector.tensor_scalar(par[:], pari[:], 1, None, ALU.bitwise_and)
    nc.vector.tensor_scalar(par2[:], par[:], -2.0, 1.0, ALU.mult, ALU.add)
    nc.vector.tensor_sub(u[:], a[:], bb[:])
    nc.vector.tensor_mul(ot[:], u[:], par2[:])
    # outT DMA issues from Pool: SP would head-block its own in-order queue
    # on the epilogue wait, gating the next tick's input loads.
    nc.gpsimd.dma_start(o_d[:], ot[:])


_SMALL = (("t", F32), ("a", F32), ("bb", F32), ("pari", I32), ("par", I32),
          ("par2", F32), ("u", F32), ("ot", FP16))


def build(loop_n: int = 0) -> bass.Bass:
    """Build the SPMD program. loop_n>0 wraps the body in a pipelined loop."""
    nc = bacc.Bacc("TRN2", target_bir_lowering=False, debug=False,
                   num_devices=N_CORES)
    # Register the clip bias as a const AP (same recipe as Bass init consts).
    bias_t = nc.alloc_sbuf_tensor("const-bias1p", [128, 1], F32)
    nc.gpsimd.memset(bias_t.ap(), BIAS1P)
    nc.const_aps.aps[(F32, BIAS1P)] = bias_t.ap()
    nc.all_engine_barrier()
    x_d = nc.dram_tensor("x", [B, I], F32, kind="ExternalInput").ap()
    m_d = nc.dram_tensor("mask", [128, I], U8, kind="ExternalInput").ap()
    o_d = nc.dram_tensor("outT", [OS, B], FP16, kind="ExternalOutput").ap()
    with tile.TileContext(nc) as tc:
        with ExitStack() as ctx:
            # One table set (natural_log_exp_and_others) covers Ln+Exp; a
            # single pre-loop load means the insertion pass adds none inside.
            set_id = [i for i, (n, _) in enumerate(
                get_activation_tables(nc.m.arch).items())
                if n == "natural_log_exp_and_others"][0]
            nc.scalar.add_instruction(mybir.InstLoadActFuncSet(
                name=nc.get_next_instruction_name(), ins=[], outs=[],
                act_func_set_id=set_id))

            sb = ctx.enter_context(tc.tile_pool(name="sb", bufs=1))
            ps = ctx.enter_context(tc.tile_pool(name="ps", bufs=1,
                                                space="PSUM"))
            if loop_n == 0:
                tl = {
                    "xc": sb.tile([128, I], F32, name="xc"),
                    "mk": sb.tile([128, I], U8, name="mk"),
                    "ax": sb.tile([128, I], F32, name="ax"),
                    "rl": sb.tile([128, I], FP16, name="rl"),
                    "rn": sb.tile([128, I], FP16, name="rn"),
                    "poL": ps.tile([128, 128], F32, name="poL"),
                    "poN": ps.tile([128, 128], F32, name="poN"),
                }
                for nm, dt in _SMALL:
                    tl[nm] = sb.tile([128, B], dt, name=nm)
                _emit_load(nc, tl["xc"], tl["mk"], x_d, m_d)
                _emit_compute(nc, tl, o_d)
            else:
                poL_ring = [ps.tile([128, 128], F32, name=f"poL{i}")
                            for i in range(4)]
                poN_ring = [ps.tile([128, 128], F32, name=f"poN{i}")
                            for i in range(4)]

                def load(pipe, iv):
                    xc = pipe.intermediate_tile([128, I], F32, name="xc")
                    mk = pipe.intermediate_tile([128, I], U8, name="mk")
                    _emit_load(nc, xc, mk, x_d, m_d)
                    return (xc, mk)

                def compute(pipe, iv, ins):
                    xc, mk = ins
                    tl = {"xc": xc, "mk": mk}
                    tl["ax"] = pipe.intermediate_tile([128, I], F32,
                                                      name="ax")
                    tl["rl"] = pipe.intermediate_tile([128, I], FP16,
                                                      name="rl")
                    tl["rn"] = pipe.intermediate_tile([128, I], FP16,
                                                      name="rn")
                    tl["poL"] = pipe.intermediate_tile(
                        [128, 128], F32, name="poL", bufs=4,
                        prealloc=poL_ring)
                    tl["poN"] = pipe.intermediate_tile(
                        [128, 128], F32, name="poN", bufs=4,
                        prealloc=poN_ring)
                    for nm, dt in _SMALL:
                        tl[nm] = pipe.intermediate_tile([128, B], dt,
                                                        name=nm)
                    _emit_compute(nc, tl, o_d)

                tc.For_i_pipelined([load, compute], 0, loop_n,
                                   unroll=8, staged_num_bufs=4,
                                   staggered_reset=True)
    nc.compile()
    return nc


def _prep_x(x: np.ndarray) -> np.ndarray:
    """Chunk-column transpose: x_cc[p, c*128+b] = x[b, c*128+p]."""
    xt = np.ascontiguousarray(x.T).reshape(N_CHUNKS, 128, 128)
    return np.ascontiguousarray(
        xt.transpose(1, 0, 2).reshape(128, I)).astype(np.float32)


def _prep_mask(shard: np.ndarray) -> np.ndarray:
    """fp8e4m3 bit pattern (1.0 -> 0x38) in chunk-column layout, uint8."""
    st = np.ascontiguousarray(shard.T).reshape(N_CHUNKS, 128, 128)
    cc = st.transpose(1, 0, 2).reshape(128, I)
    return np.ascontiguousarray((cc > 0).astype(np.uint8) * np.uint8(0x38))


_CACHE: dict = {}


def kernel(x: np.ndarray, mask: np.ndarray) -> np.ndarray:
    nc = _CACHE.get("nc")
    if nc is None:
        nc = _CACHE["nc"] = build()
    x = np.ascontiguousarray(np.asarray(x), dtype=np.float32)
    mask = np.ascontiguousarray(np.asarray(mask), dtype=np.float32)
    x_cc = _prep_x(x)
    in_maps = []
    for c in range(N_CORES):
        in_maps.append({"x": x_cc,
                        "mask": _prep_mask(mask[c * OS:(c + 1) * OS])})
    res = run_bass_kernel_spmd(nc, in_maps, list(range(N_CORES)))
    out = np.concatenate(
        [np.asarray(res.results[c]["outT"]).T for c in range(N_CORES)],
        axis=1)  # [B, O]
    return np.ascontiguousarray(out.astype(np.float32))
